# revision 2
# baseline (speedup 1.0000x reference)
"""AdaBIGGAN adaptive 1x1-conv stage, data-parallel across 8 TRN2 NeuronCores.

Math (per sample b):
    scale[b, c] = sum_k y[b, k] * Wsum[c, k] + bsum[c]
        where Wsum[c, k] = sum_j Wg_w[c*C + j, k],  bsum[c] = sum_j Wg_b[c*C + j]
    bias[b, c]  = sum_k y[b, k] * Bg_w[c, k] + Bg_b[c]
    out[b, c, :, :] = relu(h[b, c, :, :] * scale[b, c] + bias[b, c])

Sharding: batch B=32 split 4-per-core across 8 cores. The hypernet weight
Wg_w [9216, 148] is ALSO sharded: core i only loads the rows for
j in [12i, 12i+12) (687 KB instead of the full 5.45 MB), reduces them to a
partial (Wsum | bsum) [96, 149] (Wg_b's shard rides along as a 149th
column per j), and an 8-core DRAM AllReduce combines the partials. The
scale dot then uses the ones-augmented y so bsum folds into the same
accumulating multiply.
"""

import numpy as np

import concourse.bacc as bacc
import concourse.mybir as mybir
from concourse.tile import TileContext
from concourse.bass_utils import run_bass_kernel_spmd

_B, _C, _H, _W, _IN = 32, 96, 128, 128, 148
_NCORES = 8
_BL = _B // _NCORES          # 4 samples per core
_HW = _H * _W                # 16384
_ROWS = _BL * _C             # 384 rows = 3 x 128 partitions
_NPT = 3                     # row tiles of 128
_FCH = 4096                  # free-dim chunk of the h stream
_JS = _C // _NCORES          # 12 j-rows of Wg_w per core
_IA = _IN + 1                # 149: k columns + folded additive constant
_F32 = mybir.dt.float32

LAST_RESULTS = None


def _segments(r):
    """Flat rows [128r, 128r+128) split at batch boundaries -> (p0, c0, n)."""
    segs = []
    p = 0
    while p < 128:
        f = r * 128 + p
        c = f % _C
        n = min(128 - p, _C - c)
        segs.append((p, c, n))
        p += n
    return segs


def _build():
    nc = bacc.Bacc(None, num_devices=_NCORES)
    h = nc.declare_dram_parameter("h", [_ROWS, _HW], _F32, isOutput=False)
    wgp = nc.declare_dram_parameter("wgp", [_C, _JS * _IA], _F32, isOutput=False)
    yaf = nc.declare_dram_parameter("yaf", [_ROWS, _IA], _F32, isOutput=False)
    bwf = nc.declare_dram_parameter("bwf", [_ROWS, _IA], _F32, isOutput=False)
    out = nc.declare_dram_parameter("out", [_ROWS, _HW], _F32, isOutput=True)

    with TileContext(nc) as tc:
        with (
            tc.tile_pool(name="hyper", bufs=1) as hp,
            tc.tile_pool(name="stream", bufs=8) as sp,
            tc.tile_pool(name="dram", bufs=1, space="DRAM") as dp,
        ):
            # --- hypernet loads (gpsimd ring; h stream owns sync/scalar) -----
            wg_t = hp.tile([_C, _JS * _IA], _F32)
            nc.gpsimd.dma_start(out=wg_t[:], in_=wgp[:])
            ya_t, bw_t = [], []
            for r in range(_NPT):
                yt = hp.tile([128, _IA], _F32, tag=f"ya{r}")
                nc.gpsimd.dma_start(out=yt[:], in_=yaf[r * 128:(r + 1) * 128, :])
                ya_t.append(yt)
                bt = hp.tile([128, _IA], _F32, tag=f"bw{r}")
                nc.gpsimd.dma_start(out=bt[:], in_=bwf[r * 128:(r + 1) * 128, :])
                bw_t.append(bt)

            # --- bias side: zero cross-core dependencies ---------------------
            bias_fl = []
            jb = hp.tile([128, _IA], _F32)
            for r in range(_NPT):
                bf = hp.tile([128, 1], _F32, tag=f"bf{r}")
                nc.vector.scalar_tensor_tensor(
                    out=jb[:], in0=bw_t[r][:], scalar=1.0, in1=ya_t[r][:],
                    op0=mybir.AluOpType.mult, op1=mybir.AluOpType.mult,
                    accum_out=bf[:],
                )
                bias_fl.append(bf)

            # --- partial (Wsum | bsum): fold the 12 local j's ----------------
            part = hp.tile([_C, _IA], _F32)
            nc.vector.tensor_reduce(
                out=part[:],
                in_=wg_t[:].rearrange("p (j l) -> p l j", j=_JS, l=_IA),
                axis=mybir.AxisListType.X,
                op=mybir.AluOpType.add,
            )
            cc_in = dp.tile([_C, _IA], _F32, tag="cc_in")
            cc_out = dp.tile([_C, _IA], _F32, tag="cc_out")
            nc.gpsimd.dma_start(out=cc_in[:], in_=part[:])
            nc.gpsimd.collective_compute(
                "AllReduce",
                mybir.AluOpType.add,
                replica_groups=[list(range(_NCORES))],
                ins=[cc_in[:].opt()],
                outs=[cc_out[:].opt()],
            )
            wsum = hp.tile([_C, _IA], _F32)
            nc.gpsimd.dma_start(out=wsum[:], in_=cc_out[:])

            # scale dots per row tile, directly in the flat layout; the
            # 1-augmented ya column picks up bsum from wsum's col 148.
            js = hp.tile([128, _IA], _F32)
            scale_fl = []
            for r in range(_NPT):
                wsr = hp.tile([128, _IA], _F32, tag=f"ws{r}")
                for (p0, c0, n) in _segments(r):
                    nc.gpsimd.dma_start(out=wsr[p0:p0 + n, :],
                                        in_=wsum[c0:c0 + n, :])
                sf = hp.tile([128, 1], _F32, tag=f"sf{r}")
                nc.vector.scalar_tensor_tensor(
                    out=js[:], in0=wsr[:], scalar=1.0, in1=ya_t[r][:],
                    op0=mybir.AluOpType.mult, op1=mybir.AluOpType.mult,
                    accum_out=sf[:],
                )
                scale_fl.append(sf)

            # --- stream h: out = relu(h * scale + bias), fused in ScalarE ----
            # loads on sync HWDGE ring, stores on scalar HWDGE ring. The last
            # row-tile's final chunk is split fine-grained so the store tail
            # drains right behind the last loads instead of lagging 2 chunks.
            plan = []
            for r in range(_NPT):
                f0 = 0
                while f0 < _HW:
                    if r == _NPT - 1 and f0 == _HW - _FCH:
                        for w in (2048, 1024, 512, 512):
                            plan.append((r, f0, w))
                            f0 += w
                    else:
                        plan.append((r, f0, _FCH))
                        f0 += _FCH
            n_chunks = len(plan)
            for ci, (r, f0, w) in enumerate(plan):
                rows = slice(r * 128, (r + 1) * 128)
                t = sp.tile([128, _FCH], _F32, tag="st")
                # early loads also ride the (still store-free) scalar ring;
                # the final stores also ride the (by then load-free) sync
                # ring, so the drain uses both rings
                ld = nc.scalar if ci in (1, 3, 5) else nc.sync
                ld.dma_start(out=t[:, :w], in_=h[rows, f0:f0 + w])
                nc.scalar.activation(
                    out=t[:, :w], in_=t[:, :w],
                    func=mybir.ActivationFunctionType.Relu,
                    bias=bias_fl[r][:],
                    scale=scale_fl[r][:],
                )
                st = nc.sync if ci >= n_chunks - 5 else nc.scalar
                st.dma_start(out=out[rows, f0:f0 + w], in_=t[:, :w])
    nc.finalize()
    return nc


def kernel(h, y, Wg_w, Wg_b, Bg_w, Bg_b):
    global LAST_RESULTS
    h = np.ascontiguousarray(np.asarray(h), np.float32)
    y = np.ascontiguousarray(np.asarray(y), np.float32)
    Wg_w = np.ascontiguousarray(np.asarray(Wg_w), np.float32)
    Wg_b = np.ascontiguousarray(np.asarray(Wg_b), np.float32)
    Bg_w = np.ascontiguousarray(np.asarray(Bg_w), np.float32)
    Bg_b = np.ascontiguousarray(np.asarray(Bg_b), np.float32)

    nc = _build()
    # per-core Wg shard: [c, (j' in core's 12-slice, [k cols | Wg_b])]
    w3 = Wg_w.reshape(_C, _C, _IN)                      # [c, j, k]
    b2 = Wg_b.reshape(_C, _C, 1)                        # [c, j, 1]
    wgb = np.concatenate([w3, b2], axis=2)              # [c, j, 149]
    # [Bg_w | Bg_b] rows tiled to the flat [b*C + c] layout
    bw_aug = np.concatenate([Bg_w, Bg_b.reshape(_C, 1)], 1)
    bwf_r = np.ascontiguousarray(np.tile(bw_aug, (_BL, 1)))

    in_maps = []
    for i in range(_NCORES):
        hs = h[i * _BL:(i + 1) * _BL].reshape(_ROWS, _HW)
        ys = y[i * _BL:(i + 1) * _BL]          # [4, 148]
        y_aug = np.concatenate([ys, np.ones((_BL, 1), np.float32)], 1)
        wgp_i = wgb[:, i * _JS:(i + 1) * _JS, :].reshape(_C, _JS * _IA)
        in_maps.append({
            "h": np.ascontiguousarray(hs),
            "wgp": np.ascontiguousarray(wgp_i),
            "yaf": np.ascontiguousarray(np.repeat(y_aug, _C, axis=0)),
            "bwf": bwf_r,
        })

    res = run_bass_kernel_spmd(nc, in_maps, core_ids=list(range(_NCORES)))
    LAST_RESULTS = res
    outs = [r["out"].reshape(_BL, _C, _H, _W) for r in res.results]
    return np.concatenate(outs, axis=0)


# revision 3
# speedup vs baseline: 1.3129x; 1.3129x over previous
"""AdaBIGGAN adaptive 1x1-conv stage, data-parallel across 8 TRN2 NeuronCores.

Math (per sample b):
    scale[b, c] = sum_k y[b, k] * Wsum[c, k] + bsum[c]
        where Wsum[c, k] = sum_j Wg_w[c*C + j, k],  bsum[c] = sum_j Wg_b[c*C + j]
    bias[b, c]  = sum_k y[b, k] * Bg_w[c, k] + Bg_b[c]
    out[b, c, :, :] = relu(h[b, c, :, :] * scale[b, c] + bias[b, c])

Sharding: batch B=32 split 4-per-core across 8 cores; hypernet replicated.

The hypernet weight Wg_w [9216, 148] only ever enters through its j-fold
Wsum, so it is shipped as bf16 (2.73 MB instead of 5.46 MB; ~0.3% error on
Wsum, way inside the 2e-2 gate) laid out [c, (k-major, j)] so the fold is
one contiguous-stride DVE reduce. Wg_b rides along as a 149th k-column and
the scale dot uses the ones-augmented y, so bsum folds into the same
accumulating multiply. The small f32 y/Bg tables are packed into a single
[128, 894] tensor (one DMA, big descriptors) because hundreds of sub-KB
descriptors round-robin one-per-slot against the h stream's 16 KB
descriptors and would otherwise straggle for ~40 us.
"""

import numpy as np
import ml_dtypes

import concourse.bacc as bacc
import concourse.mybir as mybir
from concourse.tile import TileContext
from concourse.bass_utils import run_bass_kernel_spmd

_B, _C, _H, _W, _IN = 32, 96, 128, 128, 148
_NCORES = 8
_BL = _B // _NCORES          # 4 samples per core
_HW = _H * _W                # 16384
_ROWS = _BL * _C             # 384 rows = 3 x 128 partitions
_NPT = 3                     # row tiles of 128
_FCH = 4096                  # free-dim chunk of the h stream
_IA = _IN + 1                # 149: k columns + folded additive constant
_F32 = mybir.dt.float32
_BF16 = mybir.dt.bfloat16

LAST_RESULTS = None


def _segments(r):
    """Flat rows [128r, 128r+128) split at batch boundaries -> (p0, c0, n)."""
    segs = []
    p = 0
    while p < 128:
        f = r * 128 + p
        c = f % _C
        n = min(128 - p, _C - c)
        segs.append((p, c, n))
        p += n
    return segs


def _build():
    nc = bacc.Bacc(None, num_devices=_NCORES)
    h = nc.declare_dram_parameter("h", [_ROWS, _HW], _F32, isOutput=False)
    wgb = nc.declare_dram_parameter("wgb", [_C, _IA * _C], _BF16, isOutput=False)
    tab = nc.declare_dram_parameter("tab", [128, 6 * _IA], _F32, isOutput=False)
    out = nc.declare_dram_parameter("out", [_ROWS, _HW], _F32, isOutput=True)

    with TileContext(nc) as tc:
        with (
            tc.tile_pool(name="hyper", bufs=1) as hp,
            tc.tile_pool(name="stream", bufs=8) as sp,
        ):
            # --- hypernet loads: 2 DMAs, big descriptors, gpsimd ring --------
            wg_t = hp.tile([_C, _IA * _C], _BF16)
            nc.gpsimd.dma_start(out=wg_t[:], in_=wgb[:])
            tab_t = hp.tile([128, 6 * _IA], _F32)
            nc.gpsimd.dma_start(out=tab_t[:], in_=tab[:])
            ya_t = [tab_t[:, r * _IA:(r + 1) * _IA] for r in range(_NPT)]
            bw_t = [tab_t[:, (3 + r) * _IA:(4 + r) * _IA] for r in range(_NPT)]

            # --- bias side -----------------------------------------------
            bias_fl = []
            jb = hp.tile([128, _IA], _F32)
            for r in range(_NPT):
                bf = hp.tile([128, 1], _F32, tag=f"bf{r}")
                nc.vector.scalar_tensor_tensor(
                    out=jb[:], in0=bw_t[r], scalar=1.0, in1=ya_t[r],
                    op0=mybir.AluOpType.mult, op1=mybir.AluOpType.mult,
                    accum_out=bf[:],
                )
                bias_fl.append(bf)

            # --- (Wsum | bsum) [96, 149]: one contiguous j-fold --------------
            wsum = hp.tile([_C, _IA], _F32)
            nc.vector.tensor_reduce(
                out=wsum[:],
                in_=wg_t[:].rearrange("p (l j) -> p l j", l=_IA, j=_C),
                axis=mybir.AxisListType.X,
                op=mybir.AluOpType.add,
            )

            # scale dots per row tile, directly in the flat layout; the
            # 1-augmented ya column picks up bsum from wsum's col 148.
            js = hp.tile([128, _IA], _F32)
            scale_fl = []
            for r in range(_NPT):
                wsr = hp.tile([128, _IA], _F32, tag=f"ws{r}")
                for (p0, c0, n) in _segments(r):
                    nc.gpsimd.dma_start(out=wsr[p0:p0 + n, :],
                                        in_=wsum[c0:c0 + n, :])
                sf = hp.tile([128, 1], _F32, tag=f"sf{r}")
                nc.vector.scalar_tensor_tensor(
                    out=js[:], in0=wsr[:], scalar=1.0, in1=ya_t[r],
                    op0=mybir.AluOpType.mult, op1=mybir.AluOpType.mult,
                    accum_out=sf[:],
                )
                scale_fl.append(sf)

            # --- stream h: out = relu(h * scale + bias), fused in ScalarE ----
            # loads on sync HWDGE ring, stores on scalar HWDGE ring. The last
            # row-tile's final chunk is split fine-grained so the store tail
            # drains right behind the last loads instead of lagging 2 chunks.
            plan = []
            for r in range(_NPT):
                f0 = 0
                while f0 < _HW:
                    if r == _NPT - 1 and f0 == _HW - _FCH:
                        for w in (2048, 1024, 512, 512):
                            plan.append((r, f0, w))
                            f0 += w
                    else:
                        plan.append((r, f0, _FCH))
                        f0 += _FCH
            n_chunks = len(plan)
            for ci, (r, f0, w) in enumerate(plan):
                rows = slice(r * 128, (r + 1) * 128)
                t = sp.tile([128, _FCH], _F32, tag="st")
                # early loads also ride the (still store-free) scalar ring;
                # the final stores also ride the (by then load-free) sync
                # ring, so the drain uses both rings
                ld = nc.scalar if ci in (1, 3, 5) else nc.sync
                ld.dma_start(out=t[:, :w], in_=h[rows, f0:f0 + w])
                nc.scalar.activation(
                    out=t[:, :w], in_=t[:, :w],
                    func=mybir.ActivationFunctionType.Relu,
                    bias=bias_fl[r][:],
                    scale=scale_fl[r][:],
                )
                st = nc.sync if ci >= n_chunks - 5 else nc.scalar
                st.dma_start(out=out[rows, f0:f0 + w], in_=t[:, :w])
    nc.finalize()
    return nc


def kernel(h, y, Wg_w, Wg_b, Bg_w, Bg_b):
    global LAST_RESULTS
    h = np.ascontiguousarray(np.asarray(h), np.float32)
    y = np.ascontiguousarray(np.asarray(y), np.float32)
    Wg_w = np.ascontiguousarray(np.asarray(Wg_w), np.float32)
    Wg_b = np.ascontiguousarray(np.asarray(Wg_b), np.float32)
    Bg_w = np.ascontiguousarray(np.asarray(Bg_w), np.float32)
    Bg_b = np.ascontiguousarray(np.asarray(Bg_b), np.float32)

    nc = _build()
    # [c, (k-major | Wg_b), j] in bf16: fold over j is a contiguous reduce
    w3 = Wg_w.reshape(_C, _C, _IN)                      # [c, j, k]
    b2 = Wg_b.reshape(_C, _C, 1)                        # [c, j, 1]
    wgb_f = np.concatenate([w3, b2], 2).transpose(0, 2, 1)   # [c, 149, j]
    wgb_r = np.ascontiguousarray(
        wgb_f.reshape(_C, _IA * _C).astype(ml_dtypes.bfloat16))
    # [Bg_w | Bg_b] rows in the flat [b*C + c] layout
    bw_aug = np.concatenate([Bg_w, Bg_b.reshape(_C, 1)], 1)
    bwf_r = np.tile(bw_aug, (_BL, 1)).reshape(_NPT, 128, _IA)

    in_maps = []
    for i in range(_NCORES):
        hs = h[i * _BL:(i + 1) * _BL].reshape(_ROWS, _HW)
        ys = y[i * _BL:(i + 1) * _BL]          # [4, 148]
        y_aug = np.concatenate([ys, np.ones((_BL, 1), np.float32)], 1)
        yaf = np.repeat(y_aug, _C, axis=0).reshape(_NPT, 128, _IA)
        # pack the six [128, 149] tables into one tensor: ya0..2 | bw0..2
        tab_i = np.concatenate([yaf, bwf_r], axis=0)    # [6, 128, 149]
        tab_i = tab_i.transpose(1, 0, 2).reshape(128, 6 * _IA)
        in_maps.append({
            "h": np.ascontiguousarray(hs),
            "wgb": wgb_r,
            "tab": np.ascontiguousarray(tab_i),
        })

    res = run_bass_kernel_spmd(nc, in_maps, core_ids=list(range(_NCORES)))
    LAST_RESULTS = res
    outs = [r["out"].reshape(_BL, _C, _H, _W) for r in res.results]
    return np.concatenate(outs, axis=0)


# revision 4
# speedup vs baseline: 2.0488x; 1.5605x over previous
"""AdaBIGGAN adaptive 1x1-conv stage, data-parallel across 8 TRN2 NeuronCores.

Math (per sample b):
    scale[b, c] = sum_k y[b, k] * Wsum[c, k] + bsum[c]
        where Wsum[c, k] = sum_j Wg_w[c*C + j, k],  bsum[c] = sum_j Wg_b[c*C + j]
    bias[b, c]  = sum_k y[b, k] * Bg_w[c, k] + Bg_b[c]
    out[b, c, :, :] = relu(h[b, c, :, :] * scale[b, c] + bias[b, c])

Sharding: batch B=32 split 4-per-core across 8 cores; hypernet replicated.

Layout: channel-major [96 partitions, (sample, H*W)] so the hypernet needs
no partition shuffles at all: Wsum lands as [96, 149] straight off the
j-fold reduce, Bg rows are used as-is, and y is host-broadcast to
[96, 149] per sample. scale/bias are per-(sample) [96,1] vectors consumed
by the fused ScalarE relu over that sample's column range.

Precision: the correctness gate is rel_err < 2e-2; h and out stream as
bf16 (~0.4% L2 each, halves the dominant HBM traffic) and Wg_w/Wg_b ship
as bf16 too (they only enter through the j-fold; ~0.3% on Wsum). The
remaining f32 tables are one small packed tensor. All hypernet transfers
ride the two HWDGE rings ahead of the h chunks (the gpsimd SWDGE path
generates descriptors in software and straggles 30-60 us).
"""

import numpy as np
import ml_dtypes

import concourse.bacc as bacc
import concourse.mybir as mybir
from concourse.tile import TileContext
from concourse.bass_utils import run_bass_kernel_spmd

_B, _C, _H, _W, _IN = 32, 96, 128, 128, 148
_NCORES = 8
_BL = _B // _NCORES          # 4 samples per core
_HW = _H * _W                # 16384
_FREE = _BL * _HW            # 65536 free-dim cols per partition row
_FCH = 8192                  # free-dim chunk of the h stream (16KB bf16/desc)
_IA = _IN + 1                # 149: k columns + folded additive constant
_JW = _C * _IA               # wgb free size
_LSP = 74                    # l-split of the j-fold between the two rings
_F32 = mybir.dt.float32
_BF16 = mybir.dt.bfloat16

LAST_RESULTS = None


def _build():
    nc = bacc.Bacc(None, num_devices=_NCORES)
    h = nc.declare_dram_parameter("h", [_C, _FREE], _BF16, isOutput=False)
    wgb = nc.declare_dram_parameter("wgb", [_C, _JW], _BF16, isOutput=False)
    tab = nc.declare_dram_parameter("tab", [_C, 5 * _IA], _F32, isOutput=False)
    out = nc.declare_dram_parameter("out", [_C, _FREE], _BF16, isOutput=True)

    with TileContext(nc) as tc:
        with (
            tc.tile_pool(name="hyper", bufs=1) as hp,
            tc.tile_pool(name="stream", bufs=8) as sp,
        ):
            # --- hypernet loads: first in both HWDGE queues ------------------
            wg_t = hp.tile([_C, _JW], _BF16)
            nc.sync.dma_start(out=wg_t[:, :_LSP * _C], in_=wgb[:, :_LSP * _C])
            nc.scalar.dma_start(out=wg_t[:, _LSP * _C:], in_=wgb[:, _LSP * _C:])
            tab_t = hp.tile([_C, 5 * _IA], _F32)
            nc.sync.dma_start(out=tab_t[:], in_=tab[:])
            yb_t = [tab_t[:, b * _IA:(b + 1) * _IA] for b in range(_BL)]
            bw_v = tab_t[:, _BL * _IA:(_BL + 1) * _IA]

            # --- (Wsum | bsum) [96, 149]: contiguous j-folds, one per half ---
            wsum = hp.tile([_C, _IA], _F32)
            nc.vector.tensor_reduce(
                out=wsum[:, :_LSP],
                in_=wg_t[:, :_LSP * _C].rearrange("p (l j) -> p l j",
                                                  l=_LSP, j=_C),
                axis=mybir.AxisListType.X,
                op=mybir.AluOpType.add,
            )
            nc.vector.tensor_reduce(
                out=wsum[:, _LSP:],
                in_=wg_t[:, _LSP * _C:].rearrange("p (l j) -> p l j",
                                                  l=_IA - _LSP, j=_C),
                axis=mybir.AxisListType.X,
                op=mybir.AluOpType.add,
            )

            # --- per-sample scale/bias [96, 1] dots --------------------------
            js = hp.tile([_C, _IA], _F32)
            scale_b, bias_b = [], []
            for b in range(_BL):
                sf = hp.tile([_C, 1], _F32, tag=f"sf{b}")
                nc.vector.scalar_tensor_tensor(
                    out=js[:], in0=wsum[:], scalar=1.0, in1=yb_t[b],
                    op0=mybir.AluOpType.mult, op1=mybir.AluOpType.mult,
                    accum_out=sf[:],
                )
                scale_b.append(sf)
                bf = hp.tile([_C, 1], _F32, tag=f"bf{b}")
                nc.vector.scalar_tensor_tensor(
                    out=js[:], in0=bw_v, scalar=1.0, in1=yb_t[b],
                    op0=mybir.AluOpType.mult, op1=mybir.AluOpType.mult,
                    accum_out=bf[:],
                )
                bias_b.append(bf)

            # --- stream h: out = relu(h * scale + bias), fused in ScalarE ----
            # loads ride the sync queue, stores the scalar queue; the final
            # chunk is split fine so the store tail drains right behind the
            # last loads, and the last two stores cross onto the sync queue.
            plan = []
            for b in range(_BL):
                f0 = b * _HW
                while f0 < (b + 1) * _HW:
                    if b == _BL - 1 and f0 == (b + 1) * _HW - _FCH:
                        for w in (4096, 2048, 1024, 1024):
                            plan.append((b, f0, w))
                            f0 += w
                    else:
                        plan.append((b, f0, _FCH))
                        f0 += _FCH
            n_chunks = len(plan)
            for ci, (b, f0, w) in enumerate(plan):
                t = sp.tile([_C, _FCH], _BF16, tag="st")
                ld = nc.scalar if ci == 1 else nc.sync
                ld.dma_start(out=t[:, :w], in_=h[:, f0:f0 + w])
                nc.scalar.activation(
                    out=t[:, :w], in_=t[:, :w],
                    func=mybir.ActivationFunctionType.Relu,
                    bias=bias_b[b][:],
                    scale=scale_b[b][:],
                )
                st = nc.sync if ci >= n_chunks - 2 else nc.scalar
                st.dma_start(out=out[:, f0:f0 + w], in_=t[:, :w])
    nc.finalize()
    return nc


def kernel(h, y, Wg_w, Wg_b, Bg_w, Bg_b):
    global LAST_RESULTS
    h = np.ascontiguousarray(np.asarray(h), np.float32)
    y = np.ascontiguousarray(np.asarray(y), np.float32)
    Wg_w = np.ascontiguousarray(np.asarray(Wg_w), np.float32)
    Wg_b = np.ascontiguousarray(np.asarray(Wg_b), np.float32)
    Bg_w = np.ascontiguousarray(np.asarray(Bg_w), np.float32)
    Bg_b = np.ascontiguousarray(np.asarray(Bg_b), np.float32)

    nc = _build()
    # [c, (k-major | Wg_b), j] in bf16: fold over j is a contiguous reduce
    w3 = Wg_w.reshape(_C, _C, _IN)                      # [c, j, k]
    b2 = Wg_b.reshape(_C, _C, 1)                        # [c, j, 1]
    wgb_f = np.concatenate([w3, b2], 2).transpose(0, 2, 1)   # [c, 149, j]
    wgb_r = np.ascontiguousarray(
        wgb_f.reshape(_C, _JW).astype(ml_dtypes.bfloat16))
    bw_aug = np.concatenate([Bg_w, Bg_b.reshape(_C, 1)], 1)  # [96, 149]

    in_maps = []
    for i in range(_NCORES):
        hs = h[i * _BL:(i + 1) * _BL]                   # [4, 96, 128, 128]
        hs = hs.reshape(_BL, _C, _HW).transpose(1, 0, 2).reshape(_C, _FREE)
        ys = y[i * _BL:(i + 1) * _BL]                   # [4, 148]
        y_aug = np.concatenate([ys, np.ones((_BL, 1), np.float32)], 1)
        yb = np.repeat(y_aug[:, None, :], _C, axis=1)   # [4, 96, 149]
        tab_i = np.concatenate(
            [yb.transpose(1, 0, 2).reshape(_C, _BL * _IA), bw_aug], axis=1)
        in_maps.append({
            "h": np.ascontiguousarray(hs.astype(ml_dtypes.bfloat16)),
            "wgb": wgb_r,
            "tab": np.ascontiguousarray(tab_i),
        })

    res = run_bass_kernel_spmd(nc, in_maps, core_ids=list(range(_NCORES)))
    LAST_RESULTS = res
    outs = [
        r["out"].astype(np.float32).reshape(_C, _BL, _HW)
        .transpose(1, 0, 2).reshape(_BL, _C, _H, _W)
        for r in res.results
    ]
    return np.concatenate(outs, axis=0)
